# revision 6
# baseline (speedup 1.0000x reference)
"""BinaryTreeLSTM forward on 8 Trainium2 NeuronCores.

Strategy
--------
Data-parallel over the leaf axis: each of the 8 cores takes a contiguous
block of 2^15 = 32768 leaves and reduces its subtree through level 6
(512 nodes) on-chip; the host finishes the latency-bound tail (remaining
local levels plus the 3 cross-core levels, 4095 of 262143 nodes, ~1.6%
of FLOPs) in fp32 during gather/unshard.

Layout: feature-on-partition. Leaves are permuted host-side by 15-bit
bit-reversal so at every level left children are the first half of the
node axis and right children the second half. Level buffers are shaped
[128, 2, X] (kt = left/right block) so a level's gate GEMM can consume
both children through one strided access pattern.

Engine split (the scalar/ACT engine is the roofline at ~242us busy):
 - PE: leaf + levels 3-6 GEMMs in bf16; levels 1-2 via fp8e4m3
   DoubleRow matmuls (K=256 contraction in one instruction at 0.5
   cyc/row) -- cuts PE cycles ~2x so the PE p-state clock throttle can
   no longer make PE the critical path.
 - ACT: all sigmoid/tanh exact, per-gate ops over [128, 2048] spans.
 - DVE: gate products (bf16 2x rate), leaf c/h, h=o*tanh(c) writes
   (fp8 out for h feeding the fp8 levels).
 - GpSimd: the two c' accumulation adds (otherwise idle engine).

Precision (validated in numpy emulation against the fp32 reference):
bf16 everywhere + fp8 gate GEMMs at levels 1-2 => ~9.1e-3 rel err.
"""

import os
import sys

import numpy as np

sys.path.insert(0, "/opt/trn_rl_repo")

import ml_dtypes

N_CORES = 8
IN_DIM = 128
MEM = 128
L_GLOBAL = 262144
L = L_GLOBAL // N_CORES  # 32768 leaves per core
LOCAL_DEPTH = 15
DEVICE_DEPTH = 6  # device reduces to 512 nodes/core; host does the rest
F = 2048  # chunk size along the node axis
FP8_LEVELS = (1, 2)  # gate GEMMs in fp8 DoubleRow at these levels

_STATE = {}

LAST_EXEC_NS = None
LAST_RESULTS = None


def _build_module():
    import concourse.bacc as bacc
    import concourse.mybir as mybir
    import concourse.tile as tile

    bf = mybir.dt.bfloat16
    f8 = mybir.dt.float8e4
    f32 = mybir.dt.float32
    AF = mybir.ActivationFunctionType
    DR = mybir.MatmulPerfMode.DoubleRow

    nc = bacc.Bacc(
        "TRN2",
        target_bir_lowering=False,
        debug=False,
        enable_asserts=False,
    )

    xT = nc.dram_tensor("xt", [128, L], bf, kind="ExternalInput").ap()
    wcx = nc.dram_tensor("wcx", [128, 128], bf, kind="ExternalInput").ap()
    wox = nc.dram_tensor("wox", [128, 128], bf, kind="ExternalInput").ap()
    wl = nc.dram_tensor("wl", [128, 640], bf, kind="ExternalInput").ap()
    wr = nc.dram_tensor("wr", [128, 640], bf, kind="ExternalInput").ap()
    # fp8 DoubleRow weights: per gate g, [:, g, 0, :] = Wl[g].T, [:, g, 1, :] = Wr[g].T
    w8 = nc.dram_tensor("w8", [128, 5, 2, 128], f8, kind="ExternalInput").ap()
    # bias columns: 0=bcx, 1=box, 2..6 = (bl+br)[gate] for gates i,lf,rf,o,u
    bv = nc.dram_tensor("bv", [128, 7], f32, kind="ExternalInput").ap()
    NOUT = L >> DEVICE_DEPTH
    out = nc.dram_tensor("out", [128, 2 * NOUT], f32, kind="ExternalOutput").ap()

    with tile.TileContext(nc) as tc:
        with (
            tc.tile_pool(name="const", bufs=1) as cpool,
            tc.tile_pool(name="levels", bufs=1) as lpool,
            tc.tile_pool(name="work", bufs=2) as wpool,
            tc.tile_pool(name="psum", bufs=2, space="PSUM") as ppool,
        ):
            wcx_t = cpool.tile([128, 128], bf, name="wcx_t")
            nc.sync.dma_start(wcx_t, wcx)
            wox_t = cpool.tile([128, 128], bf, name="wox_t")
            nc.sync.dma_start(wox_t, wox)
            wl_t = cpool.tile([128, 640], bf, name="wl_t")
            nc.sync.dma_start(wl_t, wl)
            wr_t = cpool.tile([128, 640], bf, name="wr_t")
            nc.sync.dma_start(wr_t, wr)
            w8_t = cpool.tile([128, 5, 2, 128], f8, name="w8_t")
            nc.sync.dma_start(w8_t, w8)
            bias_t = cpool.tile([128, 7], f32, name="bias_t")
            nc.sync.dma_start(bias_t, bv)

            # level buffers, [128, 2, half] (kt-major: left block then right
            # block); alternate tags so level k+2 reuses level k's slot.
            cb = {}
            hb = {}
            for k in range(1, DEVICE_DEPTH):
                half = L >> (k + 1)  # (L>>k)/2
                hdt = f8 if (k + 1) in FP8_LEVELS else bf
                pad_c = [128, 2, L >> (2 if k % 2 == 0 else 1) >> 1]
                cb[k] = lpool.tile(
                    [128, 2, half], bf, name=f"c_lvl{k}", tag=f"c_ab{k % 2}",
                    padded_shape=pad_c,
                )
                # pad h slots to 16KB/partition equivalents per parity
                if k % 2 == 1:  # h1(f8,16KB), h3(bf,8KB), h5(bf,2KB)
                    pad_n = 8192 if hdt == f8 else 4096
                else:  # h2(bf,16KB), h4(bf,4KB)
                    pad_n = 4096
                hb[k] = lpool.tile(
                    [128, 2, half], hdt, name=f"h_lvl{k}", tag=f"h_ab{k % 2}",
                    padded_shape=[128, 2, pad_n],
                )
            oc = lpool.tile([128, NOUT], f32, name="oc")
            oh = lpool.tile([128, NOUT], f32, name="oh")

            # p-state warmers (off by default)
            DUMMY_N = 0

            def mm_warm(gp, wtile, rhs, s, e):
                n = min(DUMMY_N, e - s)
                if n:
                    nc.tensor.matmul(
                        gp[:, s : s + n], wtile, rhs[:, s : s + n],
                        start=True, stop=True, skip_group_check=True,
                    )

            def mm_single(gp, wtile, rhs, f):
                """out = w.T@rhs (bf16) in N<=512 pieces"""
                for s in range(0, f, 512):
                    e = min(s + 512, f)
                    nc.tensor.matmul(
                        gp[:, s:e], wtile, rhs[:, s:e], start=True, stop=True
                    )

            def mm_pair_bf16(gp, g, lh, rh, f):
                """out = wl.T@lh + wr.T@rh (bf16, accumulate in PSUM)"""
                wls = wl_t[:, g * 128 : (g + 1) * 128]
                wrs = wr_t[:, g * 128 : (g + 1) * 128]
                for s in range(0, f, 512):
                    e = min(s + 512, f)
                    nc.tensor.matmul(
                        gp[:, s:e], wls, lh[:, s:e], start=True, stop=False
                    )
                    nc.tensor.matmul(
                        gp[:, s:e], wrs, rh[:, s:e], start=False, stop=True
                    )

            def mm_pair_fp8(gp, g, rhs2, f):
                """out = wl.T@lh + wr.T@rh via one fp8 DoubleRow matmul per
                512-piece; rhs2 is a [128, 2, f] AP (kt = left/right)."""
                for s in range(0, f, 512):
                    e = min(s + 512, f)
                    nc.tensor.matmul(
                        gp[:, s:e], w8_t[:, g], rhs2[:, :, s:e],
                        start=True, stop=True, perf_mode=DR,
                    )

            # ---- pending h spans: tanh(c')*o applied in batched passes ----
            # For level 1, o-gates live in per-chunk og1 temps (h1 is fp8, so
            # o must not round-trip through fp8 before the multiply); levels
            # >= 2 store o in-place in hb and multiply in-place.
            og1_tiles = {}
            pending = []

            def emit_h_span(k, s, ln):
                half = L >> (k + 1)
                kt, off = (0, s) if s < half else (1, s - half)
                assert off + ln <= half
                csl = cb[k][:, kt, off : off + ln]
                tcy = wpool.tile([128, ln], bf, name="tcy", tag="tcy")
                nc.scalar.activation(tcy, csl, AF.Tanh)
                hsl = hb[k][:, kt, off : off + ln]
                if k == 1:
                    og = og1_tiles.pop(s)
                    nc.vector.tensor_mul(hsl, og, tcy)
                else:
                    nc.vector.tensor_mul(hsl, hsl, tcy)

            def emit_pending_one():
                if pending:
                    emit_h_span(*pending.pop(0))

            def flush_pending():
                while pending:
                    emit_h_span(*pending.pop(0))

            def h_pairs(X):
                HF = F
                if X >= 2 * HF:
                    return [(s, X // 2 + s, HF) for s in range(0, X // 2, HF)]
                return [(0, X // 2, X // 2)] if X >= 2 else [(0, 0, X)]

            # ---- per-level chunk emission ----
            def emit_level_chunk(k, j, f, rhs2, lc, rc, dst_c, dst_og, og_key):
                """gates + assembly for parents [j, j+f) of level k.
                rhs2: [128, 2, f] AP of children h; lc/rc: [128, f] bf16 APs;
                dst_c / dst_og: output APs (c', o-gate store)."""
                gps = []
                for g in range(5):
                    gp = ppool.tile([128, f], f32, name=f"g{g}", tag="ps")
                    if k in FP8_LEVELS:
                        mm_pair_fp8(gp, g, rhs2, f)
                    else:
                        mm_pair_bf16(gp, g, rhs2[:, 0], rhs2[:, 1], f)
                    gps.append(gp)
                it = wpool.tile([128, f], bf, name="it", tag="it")
                nc.scalar.activation(it, gps[0], AF.Sigmoid, bias=bias_t[:, 2:3])
                lf_ = wpool.tile([128, f], bf, name="lf_", tag="lf_")
                nc.scalar.activation(lf_, gps[1], AF.Sigmoid, bias=bias_t[:, 3:4])
                emit_pending_one()
                rf_ = wpool.tile([128, f], bf, name="rf_", tag="rf_")
                nc.scalar.activation(rf_, gps[2], AF.Sigmoid, bias=bias_t[:, 4:5])
                if og_key is not None:
                    og = wpool.tile([128, f], bf, name="og1", tag="og1", bufs=4)
                    og1_tiles[og_key] = og
                    nc.scalar.activation(og, gps[3], AF.Sigmoid, bias=bias_t[:, 5:6])
                else:
                    nc.scalar.activation(
                        dst_og, gps[3], AF.Sigmoid, bias=bias_t[:, 5:6]
                    )
                emit_pending_one()
                ut = wpool.tile([128, f], bf, name="ut", tag="ut")
                nc.scalar.activation(ut, gps[4], AF.Tanh, bias=bias_t[:, 6:7])
                # DVE: the three products (bf16 2x rate)
                nc.vector.tensor_mul(it, it, ut)     # i*u
                nc.vector.tensor_mul(lf_, lf_, lc)   # lf*lc
                nc.vector.tensor_mul(rf_, rf_, rc)   # rf*rc
                # GpSimd: the two adds
                nc.gpsimd.tensor_add(it, it, lf_)
                nc.gpsimd.tensor_add(dst_c, it, rf_)

            # ---- fused leaf + level-1 pass ----
            # L1 chunk j consumes leaf chunks (j, 16384+j). Emit L1 chunks in
            # half-alternating order so tanh(c1) span pairs (s, 8192+s) become
            # ready every 2 chunks (keeps ACT fed, og1 live-set small).
            half1 = L >> 1  # 16384 parents at level 1
            X1h = half1 // 2
            l1_order = []
            for s in range(0, X1h, F):
                l1_order += [s, X1h + s]
            pairs1 = h_pairs(half1)

            def emit_leaf_pair(j):
                """leaf transform for chunks [j, j+F) and [16384+j, ...).
                Returns (xt-free) cl2 bf16 [128,2,F], hl2 fp8 [128,2,F]."""
                xt_l = wpool.tile([128, 2, F], bf, name="xt_l", tag="xt_l", bufs=2)
                nc.sync.dma_start(xt_l[:, 0], xT[:, j : j + F])
                nc.sync.dma_start(xt_l[:, 1], xT[:, half1 + j : half1 + j + F])
                cl2 = wpool.tile([128, 2, F], bf, name="cl2", tag="cl2", bufs=2)
                hl2 = wpool.tile([128, 2, F], f8, name="hl2", tag="hl2", bufs=2)
                for kt in range(2):
                    pc = ppool.tile([128, F], f32, name="pc", tag="ps")
                    mm_single(pc, wcx_t, xt_l[:, kt], F)
                    po = ppool.tile([128, F], f32, name="po", tag="ps")
                    mm_single(po, wox_t, xt_l[:, kt], F)
                    th = wpool.tile([128, F], bf, name="th", tag="th")
                    nc.scalar.activation(th, pc, AF.Tanh, bias=bias_t[:, 0:1])
                    og = wpool.tile([128, F], bf, name="og0", tag="og0")
                    nc.scalar.activation(og, po, AF.Sigmoid, bias=bias_t[:, 1:2])
                    nc.vector.tensor_scalar_add(cl2[:, kt], pc, bias_t[:, 0:1])
                    nc.vector.tensor_mul(hl2[:, kt], og, th)
                return cl2, hl2

            hi1 = 0
            done1 = set()

            def drain1():
                nonlocal hi1
                while hi1 < len(pairs1):
                    s1, s2, ln = pairs1[hi1]
                    if not (s1 in done1 and s2 in done1):
                        break
                    pending.append((1, s1, ln))
                    pending.append((1, s2, ln))
                    hi1 += 1

            prev = None

            def l1_assembly(prev):
                (cl2, hl2), pj = prev
                half = X1h
                kt, off = (0, pj) if pj < half else (1, pj - half)
                sl = (slice(None), kt, slice(off, off + F))
                emit_level_chunk(
                    1, pj, F, hl2, cl2[:, 0], cl2[:, 1],
                    cb[1][sl], None, og_key=pj,
                )
                done1.add(pj)
                drain1()

            for j in l1_order:
                cur = (emit_leaf_pair(j), j)
                if prev is not None:
                    l1_assembly(prev)
                prev = cur
            l1_assembly(prev)
            flush_pending()

            # ---- levels 2..DEVICE_DEPTH ----
            for k in range(2, DEVICE_DEPTH + 1):
                X = L >> k  # parents at this level
                Xh = X // 2
                f = min(F, Xh)  # chunks never cross the kt half boundary
                pairs = h_pairs(X)
                hi = 0
                order = []
                for a, b in zip(range(0, Xh, f), range(Xh, X, f)):
                    order += [a, b]
                done = set()

                def span_ready(s, ln, done=done, f=f):
                    return all(q - q % f in done for q in range(s, s + ln, f))

                for j in order:
                    kt, off = (0, j) if j < Xh else (1, j - Xh)
                    lc = cb[k - 1][:, 0, j : j + f]
                    rc = cb[k - 1][:, 1, j : j + f]
                    rhs2 = hb[k - 1][:, :, j : j + f]
                    if k == DEVICE_DEPTH:
                        # assemble straight into the fp32 output tiles
                        dst_c = oc[:, j : j + f]
                        og = wpool.tile([128, f], bf, name="og6", tag="og6")
                        emit_level_chunk(
                            k, j, f, rhs2, lc, rc, dst_c, og, og_key=None
                        )
                        tcy = wpool.tile([128, f], bf, name="tcy6", tag="tcy")
                        nc.scalar.activation(tcy, dst_c, AF.Tanh)
                        nc.vector.tensor_mul(oh[:, j : j + f], og, tcy)
                    else:
                        dst_c = cb[k][:, kt, off : off + f]
                        dst_og = hb[k][:, kt, off : off + f]
                        emit_level_chunk(
                            k, j, f, rhs2, lc, rc, dst_c, dst_og, og_key=None
                        )
                        done.add(j)
                        while hi < len(pairs):
                            s1, s2, ln = pairs[hi]
                            if not (span_ready(s1, ln) and span_ready(s2, ln)):
                                break
                            pending.append((k, s1, ln))
                            if s2 > s1:
                                pending.append((k, s2, ln))
                            hi += 1
                flush_pending()

            nc.sync.dma_start(out[:, 0:NOUT], oc)
            nc.sync.dma_start(out[:, NOUT : 2 * NOUT], oh)

    nc.compile()
    return nc


def _get_module():
    if "nc" not in _STATE:
        _STATE["nc"] = _build_module()
    return _STATE["nc"]


def _bitrev_perm(bits):
    n = 1 << bits
    i = np.arange(n, dtype=np.int64)
    r = np.zeros_like(i)
    for b in range(bits):
        r |= ((i >> b) & 1) << (bits - 1 - b)
    return r


def _run_spmd(nc, in_maps, trace):
    """Run via run_bass_kernel_spmd; with trace, drive NTFF profiling
    directly (this image's antenv lacks axon_hooks, so the built-in
    trace path is unavailable)."""
    from concourse import bass_utils

    if not trace:
        res = bass_utils.run_bass_kernel_spmd(
            nc, in_maps, core_ids=list(range(N_CORES))
        )
        return res.results, None, None

    import glob
    import tempfile

    from concourse import bass2jax

    hook = None
    try:
        from trn_agent_boot.trn_boot import _ntff_profile_via_ctypes

        hook = _ntff_profile_via_ctypes("/opt/axon/libaxon_pjrt.so")
    except Exception as e:  # noqa: BLE001
        print(f"trace hook unavailable: {e}")
    if hook is None:
        res = bass_utils.run_bass_kernel_spmd(
            nc, in_maps, core_ids=list(range(N_CORES))
        )
        return res.results, None, None

    neff_dir = tempfile.mkdtemp(prefix="bk_prof_")
    with hook(neff_dir, [0]):
        results = bass2jax.run_bass_via_pjrt(nc, in_maps, n_cores=N_CORES)

    exec_ns = None
    trace_path = None
    ntffs = glob.glob(os.path.join(neff_dir, "*_body*.ntff"))
    if ntffs:
        try:
            import gauge.profiler as gp
            from concourse._compat import FishPath

            profile = gp.Profile(
                profile_path=FishPath(neff_dir),
                kernel_dev_mode=True,
                profile_on_exit=False,
                bass_kernel=nc.m,
                offline_processing=True,
                fname="*_body*",
            )
            prs = profile.to_perfetto(model_index=(0,))
            if prs:
                exec_ns = prs[0].exec_time_ns
                trace_path = prs[0].trace_path
        except Exception as e:  # noqa: BLE001
            print(f"ntff processing failed: {e}")
    else:
        print(f"no NTFF produced in {neff_dir}")
    return results, exec_ns, (neff_dir, trace_path)


def kernel(inputs, Wcx, bcx, Wox, box, Wl, bl, Wr, br):
    global LAST_EXEC_NS, LAST_RESULTS

    bf16 = ml_dtypes.bfloat16
    fp8 = ml_dtypes.float8_e4m3fn
    x = np.asarray(inputs, np.float32)
    Wcx = np.asarray(Wcx, np.float32)
    bcx = np.asarray(bcx, np.float32)
    Wox = np.asarray(Wox, np.float32)
    box = np.asarray(box, np.float32)
    Wl = np.asarray(Wl, np.float32)
    bl = np.asarray(bl, np.float32)
    Wr = np.asarray(Wr, np.float32)
    br = np.asarray(br, np.float32)

    nc = _get_module()

    WcxT = np.ascontiguousarray(Wcx.T).astype(bf16)
    WoxT = np.ascontiguousarray(Wox.T).astype(bf16)
    WlT = np.ascontiguousarray(
        np.concatenate([Wl[g].T for g in range(5)], axis=1)
    ).astype(bf16)  # [128, 640]
    WrT = np.ascontiguousarray(
        np.concatenate([Wr[g].T for g in range(5)], axis=1)
    ).astype(bf16)
    # [128, 5, 2, 128]: (k, gate, {l,r}, m)
    W8 = np.ascontiguousarray(
        np.stack(
            [np.stack([Wl[g].T, Wr[g].T], axis=1) for g in range(5)], axis=1
        )
    ).astype(fp8)
    bg = bl + br  # [5, 128]
    bvec = np.stack(
        [bcx, box, bg[0], bg[1], bg[2], bg[3], bg[4]], axis=1
    ).astype(np.float32)  # [128, 7]

    perm = _bitrev_perm(LOCAL_DEPTH)
    in_maps = []
    for m in range(N_CORES):
        shard = x[m * L : (m + 1) * L][perm]  # [L, 128]
        xt = np.ascontiguousarray(shard.T).astype(bf16)  # [128, L]
        in_maps.append(
            dict(xt=xt, wcx=WcxT, wox=WoxT, wl=WlT, wr=WrT, w8=W8, bv=bvec)
        )

    trace = bool(int(os.environ.get("BK_TRACE", "0")))
    results, exec_ns, trace_info = _run_spmd(nc, in_maps, trace)
    LAST_EXEC_NS = exec_ns
    LAST_RESULTS = trace_info

    bias5 = bg[:, None, :]  # [5, 1, 128]
    sig = lambda v: 1.0 / (1.0 + np.exp(-v))

    def level_np(c, h, lc, rc, lh, rh):
        g = (
            np.einsum("xm,gnm->gxn", lh, Wl)
            + np.einsum("xm,gnm->gxn", rh, Wr)
            + bias5
        )
        i = sig(g[0])
        lf = sig(g[1])
        rf = sig(g[2])
        o = sig(g[3])
        u = np.tanh(g[4])
        c = i * u + lf * lc + rf * rc
        h = o * np.tanh(c)
        return c, h

    # finish remaining local levels on host (bit-reversed halves pairing),
    # then the cross-core levels (adjacent pairing)
    NOUT = L >> DEVICE_DEPTH
    roots_c, roots_h = [], []
    for o in results:
        om = np.asarray(o["out"], np.float32)
        c = om[:, 0:NOUT].T  # [NOUT, 128]
        h = om[:, NOUT : 2 * NOUT].T
        while c.shape[0] > 1:
            half = c.shape[0] // 2
            c, h = level_np(c, h, c[:half], c[half:], h[:half], h[half:])
        roots_c.append(c[0])
        roots_h.append(h[0])
    c = np.stack(roots_c)  # [8, 128]
    h = np.stack(roots_h)
    while c.shape[0] > 1:
        c, h = level_np(c, h, c[0::2], c[1::2], h[0::2], h[1::2])
    return np.asarray(c, np.float32), np.asarray(h, np.float32)


# revision 8
# speedup vs baseline: 1.1877x; 1.1877x over previous
"""BinaryTreeLSTM forward on 8 Trainium2 NeuronCores.

Strategy
--------
Data-parallel over the leaf axis: each of the 8 cores takes a contiguous
block of 2^15 = 32768 leaves and reduces its subtree through level 5
(1024 nodes) on-chip; the host finishes the latency-bound tail (the
remaining local levels plus the 3 cross-core levels, ~8k of 262143
nodes) in fp32 during gather/unshard.

Layout: feature-on-partition. Leaves are permuted host-side by 15-bit
bit-reversal so at every level left children are the first half of the
node axis and right children the second half.

Engine split (the scalar/ACT engine is the roofline at ~240us busy):
 - PE: leaf + levels 1-2 gate GEMMs as fp8e4m3 DoubleRow matmuls
   (K=2x contraction in one instruction at 0.5 cyc/row); levels 3-5
   bf16. Cuts PE cycles ~2.5x so the PE p-state clock throttle cannot
   make PE the critical path.
 - ACT: all sigmoid/tanh exact, per-gate ops over [128, <=2048] spans.
 - DVE: gate products (bf16 2x rate), leaf c/h, h=o*tanh(c) writes
   (fp8 out for h feeding the fp8 levels), c' adds at the top levels.
 - GpSimd: the two c' accumulation adds at the wide levels.

Precision (validated in numpy emulation against the fp32 reference):
bf16 + fp8 leaf/L1/L2 GEMMs => ~8.6e-3 rel err (gate: 2e-2).
"""

import os
import sys

import numpy as np

sys.path.insert(0, "/opt/trn_rl_repo")

import ml_dtypes

N_CORES = 8
IN_DIM = 128
MEM = 128
L_GLOBAL = 262144
L = L_GLOBAL // N_CORES  # 32768 leaves per core
LOCAL_DEPTH = 15
DEVICE_DEPTH = 5  # device reduces to 1024 nodes/core; host does the rest
F = 2048  # chunk size along the node axis
FP8_LEVELS = (1, 2)  # gate GEMMs in fp8 DoubleRow at these levels
POOL_ADD_LEVELS = (1, 2, 3)  # c' adds on GpSimd here, on DVE above

_STATE = {}

LAST_EXEC_NS = None
LAST_RESULTS = None


def _build_module():
    import concourse.bacc as bacc
    import concourse.mybir as mybir
    import concourse.tile as tile

    bf = mybir.dt.bfloat16
    f8 = mybir.dt.float8e4
    f32 = mybir.dt.float32
    AF = mybir.ActivationFunctionType
    DR = mybir.MatmulPerfMode.DoubleRow

    nc = bacc.Bacc(
        "TRN2",
        target_bir_lowering=False,
        debug=False,
        enable_asserts=False,
    )

    # x8: leaf inputs, feature dim split across DoubleRow k-tiles:
    # x8[p, t, n] = x_bitrev[n, 64*t + p]
    x8 = nc.dram_tensor("x8", [64, 2, L], f8, kind="ExternalInput").ap()
    # leaf weights [64, 2, 128]: [p, t, m] = W.T[64*t + p, m]
    wcx8 = nc.dram_tensor("wcx8", [64, 2, 128], f8, kind="ExternalInput").ap()
    wox8 = nc.dram_tensor("wox8", [64, 2, 128], f8, kind="ExternalInput").ap()
    wl = nc.dram_tensor("wl", [128, 640], bf, kind="ExternalInput").ap()
    wr = nc.dram_tensor("wr", [128, 640], bf, kind="ExternalInput").ap()
    # fp8 DoubleRow gate weights: [:, g, 0, :] = Wl[g].T, [:, g, 1, :] = Wr[g].T
    w8 = nc.dram_tensor("w8", [128, 5, 2, 128], f8, kind="ExternalInput").ap()
    # bias columns: 0=bcx, 1=box, 2..6 = (bl+br)[gate] for gates i,lf,rf,o,u
    bv = nc.dram_tensor("bv", [128, 7], f32, kind="ExternalInput").ap()
    NOUT = L >> DEVICE_DEPTH
    out = nc.dram_tensor("out", [128, 2 * NOUT], f32, kind="ExternalOutput").ap()

    with tile.TileContext(nc) as tc:
        with (
            tc.tile_pool(name="const", bufs=1) as cpool,
            tc.tile_pool(name="levels", bufs=1) as lpool,
            tc.tile_pool(name="work", bufs=2) as wpool,
            tc.tile_pool(name="psum", bufs=2, space="PSUM") as ppool,
        ):
            wcx_t = cpool.tile([64, 2, 128], f8, name="wcx_t")
            nc.sync.dma_start(wcx_t, wcx8)
            wox_t = cpool.tile([64, 2, 128], f8, name="wox_t")
            nc.sync.dma_start(wox_t, wox8)
            wl_t = cpool.tile([128, 640], bf, name="wl_t")
            nc.sync.dma_start(wl_t, wl)
            wr_t = cpool.tile([128, 640], bf, name="wr_t")
            nc.sync.dma_start(wr_t, wr)
            w8_t = cpool.tile([128, 5, 2, 128], f8, name="w8_t")
            nc.sync.dma_start(w8_t, w8)
            bias_t = cpool.tile([128, 7], f32, name="bias_t")
            nc.sync.dma_start(bias_t, bv)

            # level buffers. h1 keeps the [128, 2, half] kt layout (it is the
            # fp8 DoubleRow rhs for level 2); everything else is flat.
            cb = {
                1: lpool.tile([128, L >> 1], bf, name="c1", tag="c_odd",
                              padded_shape=[128, L >> 1]),
                2: lpool.tile([128, L >> 2], bf, name="c2", tag="c_even",
                              padded_shape=[128, L >> 2]),
                3: lpool.tile([128, L >> 3], bf, name="c3", tag="c_odd",
                              padded_shape=[128, L >> 1]),
                4: lpool.tile([128, L >> 4], bf, name="c4", tag="c_even",
                              padded_shape=[128, L >> 2]),
            }
            hb = {
                1: lpool.tile([128, 2, L >> 2], f8, name="h1", tag="h_odd",
                              padded_shape=[128, 2, L >> 2]),
                2: lpool.tile([128, L >> 2], bf, name="h2", tag="h_even",
                              padded_shape=[128, L >> 2]),
                3: lpool.tile([128, L >> 3], bf, name="h3", tag="h_odd",
                              padded_shape=[128, L >> 2]),
                4: lpool.tile([128, L >> 4], bf, name="h4", tag="h_even",
                              padded_shape=[128, L >> 2]),
            }
            oc = lpool.tile([128, NOUT], f32, name="oc")
            oh = lpool.tile([128, NOUT], f32, name="oh")

            def mm_dr(gp, wtile, rhs2, f):
                """out = w.T@rhs over K=2x contraction, fp8 DoubleRow."""
                for s in range(0, f, 512):
                    e = min(s + 512, f)
                    nc.tensor.matmul(
                        gp[:, s:e], wtile, rhs2[:, :, s:e],
                        start=True, stop=True, perf_mode=DR,
                    )

            def mm_pair_bf16(gp, g, lh, rh, f):
                wls = wl_t[:, g * 128 : (g + 1) * 128]
                wrs = wr_t[:, g * 128 : (g + 1) * 128]
                for s in range(0, f, 512):
                    e = min(s + 512, f)
                    nc.tensor.matmul(
                        gp[:, s:e], wls, lh[:, s:e], start=True, stop=False
                    )
                    nc.tensor.matmul(
                        gp[:, s:e], wrs, rh[:, s:e], start=False, stop=True
                    )

            # ---- pending h spans: tanh(c')*o applied in batched passes ----
            og1_tiles = {}
            pending = []

            def emit_h_span(k, s, ln):
                if k == 1:
                    half = L >> 2
                    kt, off = (0, s) if s < half else (1, s - half)
                    csl = cb[1][:, s : s + ln]
                    tcy = wpool.tile([128, ln], bf, name="tcy", tag="tcy")
                    nc.scalar.activation(tcy, csl, AF.Tanh)
                    og = og1_tiles.pop(s)
                    nc.vector.tensor_mul(
                        hb[1][:, kt, off : off + ln], og, tcy
                    )
                else:
                    csl = cb[k][:, s : s + ln]
                    tcy = wpool.tile([128, ln], bf, name="tcy", tag="tcy")
                    nc.scalar.activation(tcy, csl, AF.Tanh)
                    hsl = hb[k][:, s : s + ln]
                    nc.vector.tensor_mul(hsl, hsl, tcy)

            def emit_pending_one():
                if pending:
                    emit_h_span(*pending.pop(0))

            def flush_pending():
                while pending:
                    emit_h_span(*pending.pop(0))

            def h_pairs(X):
                HF = F
                if X >= 2 * HF:
                    return [(s, X // 2 + s, HF) for s in range(0, X // 2, HF)]
                return [(0, X // 2, X // 2)] if X >= 2 else [(0, 0, X)]

            # ---- per-level chunk emission ----
            def emit_level_chunk(k, f, rhs_dr, lh, rh, lc, rc, dst_c, dst_og,
                                 og_key):
                gps = []
                for g in range(5):
                    gp = ppool.tile([128, f], f32, name=f"g{g}", tag="ps")
                    if k in FP8_LEVELS:
                        mm_dr(gp, w8_t[:, g], rhs_dr, f)
                    else:
                        mm_pair_bf16(gp, g, lh, rh, f)
                    gps.append(gp)
                it = wpool.tile([128, f], bf, name="it", tag="it")
                nc.scalar.activation(it, gps[0], AF.Sigmoid, bias=bias_t[:, 2:3])
                lf_ = wpool.tile([128, f], bf, name="lf_", tag="lf_")
                nc.scalar.activation(lf_, gps[1], AF.Sigmoid, bias=bias_t[:, 3:4])
                emit_pending_one()
                rf_ = wpool.tile([128, f], bf, name="rf_", tag="rf_")
                nc.scalar.activation(rf_, gps[2], AF.Sigmoid, bias=bias_t[:, 4:5])
                if og_key is not None:
                    og = wpool.tile([128, f], bf, name="og1", tag="og1", bufs=3)
                    og1_tiles[og_key] = og
                    nc.scalar.activation(og, gps[3], AF.Sigmoid, bias=bias_t[:, 5:6])
                else:
                    nc.scalar.activation(
                        dst_og, gps[3], AF.Sigmoid, bias=bias_t[:, 5:6]
                    )
                emit_pending_one()
                ut = wpool.tile([128, f], bf, name="ut", tag="ut")
                nc.scalar.activation(ut, gps[4], AF.Tanh, bias=bias_t[:, 6:7])
                nc.vector.tensor_mul(it, it, ut)     # i*u
                nc.vector.tensor_mul(lf_, lf_, lc)   # lf*lc
                nc.vector.tensor_mul(rf_, rf_, rc)   # rf*rc
                if k in POOL_ADD_LEVELS:
                    nc.gpsimd.tensor_add(it, it, lf_)
                    nc.gpsimd.tensor_add(dst_c, it, rf_)
                else:
                    nc.vector.tensor_add(it, it, lf_)
                    nc.vector.tensor_add(dst_c, it, rf_)

            # ---- fused leaf + level-1 pass ----
            half1 = L >> 1  # 16384 parents at level 1
            X1h = half1 // 2
            l1_order = []
            for s in range(0, X1h, F):
                l1_order += [s, X1h + s]
            pairs1 = h_pairs(half1)

            def emit_leaf_pair(j):
                """leaf transform for leaf chunks [j, j+F) (left children)
                and [half1+j, ...) (right children) of L1 chunk j."""
                xt_l = wpool.tile([64, 2, 2 * F], f8, name="xt_l", tag="xt_l",
                                  bufs=2)
                nc.sync.dma_start(xt_l[:, :, 0:F], x8[:, :, j : j + F])
                nc.sync.dma_start(
                    xt_l[:, :, F : 2 * F], x8[:, :, half1 + j : half1 + j + F]
                )
                cl2 = wpool.tile([128, 2, F], bf, name="cl2", tag="cl2", bufs=2)
                hl2 = wpool.tile([128, 2, F], f8, name="hl2", tag="hl2", bufs=2)
                for c in range(2):
                    xs = xt_l[:, :, c * F : (c + 1) * F]
                    pc = ppool.tile([128, F], f32, name="pc", tag="ps")
                    mm_dr(pc, wcx_t, xs, F)
                    po = ppool.tile([128, F], f32, name="po", tag="ps")
                    mm_dr(po, wox_t, xs, F)
                    th = wpool.tile([128, F], bf, name="th", tag="th")
                    nc.scalar.activation(th, pc, AF.Tanh, bias=bias_t[:, 0:1])
                    og = wpool.tile([128, F], bf, name="og0", tag="og0")
                    nc.scalar.activation(og, po, AF.Sigmoid, bias=bias_t[:, 1:2])
                    nc.vector.tensor_scalar_add(cl2[:, c], pc, bias_t[:, 0:1])
                    nc.vector.tensor_mul(hl2[:, c], og, th)
                return cl2, hl2

            hi1 = 0
            done1 = set()

            def drain1():
                nonlocal hi1
                while hi1 < len(pairs1):
                    s1, s2, ln = pairs1[hi1]
                    if not (s1 in done1 and s2 in done1):
                        break
                    pending.append((1, s1, ln))
                    pending.append((1, s2, ln))
                    hi1 += 1

            def l1_assembly(prev):
                (cl2, hl2), pj = prev
                emit_level_chunk(
                    1, F, hl2, None, None, cl2[:, 0], cl2[:, 1],
                    cb[1][:, pj : pj + F], None, og_key=pj,
                )
                done1.add(pj)
                drain1()

            prev = None
            for j in l1_order:
                if prev is not None:
                    l1_assembly(prev)
                prev = (emit_leaf_pair(j), j)
            l1_assembly(prev)
            flush_pending()

            # ---- levels 2..DEVICE_DEPTH ----
            for k in range(2, DEVICE_DEPTH + 1):
                X = L >> k  # parents at this level
                Xh = X // 2
                f = min(F, X)
                pairs = h_pairs(X)
                hi = 0
                if X // f >= 2:
                    order = []
                    for a, b in zip(range(0, Xh, f), range(Xh, X, f)):
                        order += [a, b]
                else:
                    order = [0]
                done = set()

                def span_ready(s, ln, done=done, f=f):
                    return all(q - q % f in done for q in range(s, s + ln, f))

                for j in order:
                    if k == 2:
                        rhs_dr = hb[1][:, :, j : j + f]
                        lh = rh = None
                    else:
                        rhs_dr = None
                        lh = hb[k - 1][:, j : j + f]
                        rh = hb[k - 1][:, X + j : X + j + f]
                    lc = cb[k - 1][:, j : j + f]
                    rc = cb[k - 1][:, X + j : X + j + f]
                    if k == DEVICE_DEPTH:
                        dst_c = oc[:, j : j + f]
                        og = wpool.tile([128, f], bf, name="ogN", tag="ogN")
                        emit_level_chunk(
                            k, f, rhs_dr, lh, rh, lc, rc, dst_c, og,
                            og_key=None,
                        )
                        tcy = wpool.tile([128, f], bf, name="tcyN", tag="tcy")
                        nc.scalar.activation(tcy, dst_c, AF.Tanh)
                        nc.vector.tensor_mul(oh[:, j : j + f], og, tcy)
                    else:
                        dst_c = cb[k][:, j : j + f]
                        dst_og = hb[k][:, j : j + f]
                        emit_level_chunk(
                            k, f, rhs_dr, lh, rh, lc, rc, dst_c, dst_og,
                            og_key=None,
                        )
                        done.add(j)
                        while hi < len(pairs):
                            s1, s2, ln = pairs[hi]
                            if not (span_ready(s1, ln) and span_ready(s2, ln)):
                                break
                            pending.append((k, s1, ln))
                            if s2 > s1:
                                pending.append((k, s2, ln))
                            hi += 1
                flush_pending()

            nc.sync.dma_start(out[:, 0:NOUT], oc)
            nc.sync.dma_start(out[:, NOUT : 2 * NOUT], oh)

    nc.compile()
    return nc


def _get_module():
    if "nc" not in _STATE:
        _STATE["nc"] = _build_module()
    return _STATE["nc"]


def _bitrev_perm(bits):
    n = 1 << bits
    i = np.arange(n, dtype=np.int64)
    r = np.zeros_like(i)
    for b in range(bits):
        r |= ((i >> b) & 1) << (bits - 1 - b)
    return r


def _run_spmd(nc, in_maps, trace):
    """Run via run_bass_kernel_spmd; with trace, drive NTFF profiling
    directly."""
    from concourse import bass_utils

    if not trace:
        res = bass_utils.run_bass_kernel_spmd(
            nc, in_maps, core_ids=list(range(N_CORES))
        )
        return res.results, None, None

    import glob
    import tempfile

    from concourse import bass2jax

    hook = None
    try:
        from trn_agent_boot.trn_boot import _ntff_profile_via_ctypes

        hook = _ntff_profile_via_ctypes("/opt/axon/libaxon_pjrt.so")
    except Exception as e:  # noqa: BLE001
        print(f"trace hook unavailable: {e}")
    if hook is None:
        res = bass_utils.run_bass_kernel_spmd(
            nc, in_maps, core_ids=list(range(N_CORES))
        )
        return res.results, None, None

    neff_dir = tempfile.mkdtemp(prefix="bk_prof_")
    with hook(neff_dir, [0]):
        results = bass2jax.run_bass_via_pjrt(nc, in_maps, n_cores=N_CORES)

    exec_ns = None
    trace_path = None
    ntffs = glob.glob(os.path.join(neff_dir, "*_body*.ntff"))
    if ntffs:
        try:
            import gauge.profiler as gp
            from concourse._compat import FishPath

            profile = gp.Profile(
                profile_path=FishPath(neff_dir),
                kernel_dev_mode=True,
                profile_on_exit=False,
                bass_kernel=nc.m,
                offline_processing=True,
                fname="*_body*",
            )
            prs = profile.to_perfetto(model_index=(0,))
            if prs:
                exec_ns = prs[0].exec_time_ns
                trace_path = prs[0].trace_path
        except Exception as e:  # noqa: BLE001
            print(f"ntff processing failed: {e}")
    else:
        print(f"no NTFF produced in {neff_dir}")
    return results, exec_ns, (neff_dir, trace_path)


def kernel(inputs, Wcx, bcx, Wox, box, Wl, bl, Wr, br):
    global LAST_EXEC_NS, LAST_RESULTS

    fp8 = ml_dtypes.float8_e4m3fn
    bf16 = ml_dtypes.bfloat16
    x = np.asarray(inputs, np.float32)
    Wcx = np.asarray(Wcx, np.float32)
    bcx = np.asarray(bcx, np.float32)
    Wox = np.asarray(Wox, np.float32)
    box = np.asarray(box, np.float32)
    Wl = np.asarray(Wl, np.float32)
    bl = np.asarray(bl, np.float32)
    Wr = np.asarray(Wr, np.float32)
    br = np.asarray(br, np.float32)

    nc = _get_module()

    # leaf weights [64, 2, 128]: [p, t, m] = W.T[64t+p, m]
    Wcx8 = np.ascontiguousarray(Wcx.T.reshape(2, 64, 128).transpose(1, 0, 2)).astype(fp8)
    Wox8 = np.ascontiguousarray(Wox.T.reshape(2, 64, 128).transpose(1, 0, 2)).astype(fp8)
    WlT = np.ascontiguousarray(
        np.concatenate([Wl[g].T for g in range(5)], axis=1)
    ).astype(bf16)  # [128, 640]
    WrT = np.ascontiguousarray(
        np.concatenate([Wr[g].T for g in range(5)], axis=1)
    ).astype(bf16)
    W8 = np.ascontiguousarray(
        np.stack(
            [np.stack([Wl[g].T, Wr[g].T], axis=1) for g in range(5)], axis=1
        )
    ).astype(fp8)  # [128, 5, 2, 128]
    bg = bl + br  # [5, 128]
    bvec = np.stack(
        [bcx, box, bg[0], bg[1], bg[2], bg[3], bg[4]], axis=1
    ).astype(np.float32)  # [128, 7]

    perm = _bitrev_perm(LOCAL_DEPTH)
    in_maps = []
    for m in range(N_CORES):
        shard = x[m * L : (m + 1) * L][perm]  # [L, 128]
        xt = np.ascontiguousarray(shard.T)  # [128, L] fp32
        x8v = np.ascontiguousarray(
            xt.reshape(2, 64, L).transpose(1, 0, 2)
        ).astype(fp8)  # [64, 2, L]
        in_maps.append(
            dict(x8=x8v, wcx8=Wcx8, wox8=Wox8, wl=WlT, wr=WrT, w8=W8, bv=bvec)
        )

    trace = bool(int(os.environ.get("BK_TRACE", "0")))
    results, exec_ns, trace_info = _run_spmd(nc, in_maps, trace)
    LAST_EXEC_NS = exec_ns
    LAST_RESULTS = trace_info

    bias5 = bg[:, None, :]  # [5, 1, 128]
    sig = lambda v: 1.0 / (1.0 + np.exp(-v))

    def level_np(c, h, lc, rc, lh, rh):
        g = (
            np.einsum("xm,gnm->gxn", lh, Wl)
            + np.einsum("xm,gnm->gxn", rh, Wr)
            + bias5
        )
        i = sig(g[0])
        lf = sig(g[1])
        rf = sig(g[2])
        o = sig(g[3])
        u = np.tanh(g[4])
        c = i * u + lf * lc + rf * rc
        h = o * np.tanh(c)
        return c, h

    NOUT = L >> DEVICE_DEPTH
    roots_c, roots_h = [], []
    for o in results:
        om = np.asarray(o["out"], np.float32)
        c = om[:, 0:NOUT].T  # [NOUT, 128]
        h = om[:, NOUT : 2 * NOUT].T
        while c.shape[0] > 1:
            half = c.shape[0] // 2
            c, h = level_np(c, h, c[:half], c[half:], h[:half], h[half:])
        roots_c.append(c[0])
        roots_h.append(h[0])
    c = np.stack(roots_c)  # [8, 128]
    h = np.stack(roots_h)
    while c.shape[0] > 1:
        c, h = level_np(c, h, c[0::2], c[1::2], h[0::2], h[1::2])
    return np.asarray(c, np.float32), np.asarray(h, np.float32)


# revision 15
# speedup vs baseline: 1.1894x; 1.0014x over previous
"""BinaryTreeLSTM forward on 8 Trainium2 NeuronCores.

Strategy
--------
Data-parallel over the leaf axis: each of the 8 cores takes a contiguous
block of 2^15 = 32768 leaves and reduces its subtree through level 5
(1024 nodes) on-chip; the host finishes the latency-bound tail (the
remaining local levels plus the 3 cross-core levels, ~8k of 262143
nodes) in fp32 during gather/unshard.

Layout: feature-on-partition. Leaves are permuted host-side by 15-bit
bit-reversal so at every level left children are the first half of the
node axis and right children the second half.

Engine split (the scalar/ACT engine is the roofline at ~240us busy):
 - PE: leaf + levels 1-2 gate GEMMs as fp8e4m3 DoubleRow matmuls
   (K=2x contraction in one instruction at 0.5 cyc/row); levels 3-5
   bf16. Cuts PE cycles ~2.5x so the PE p-state clock throttle cannot
   make PE the critical path.
 - ACT: all sigmoid/tanh exact, per-gate ops over [128, <=2048] spans.
 - DVE: gate products (bf16 2x rate), leaf c/h, h=o*tanh(c) writes
   (fp8 out for h feeding the fp8 levels), c' adds at the top levels.
 - GpSimd: the two c' accumulation adds at the wide levels.

Precision (validated in numpy emulation against the fp32 reference):
bf16 + fp8 leaf/L1/L2 GEMMs => ~8.6e-3 rel err (gate: 2e-2).
"""

import os
import sys

import numpy as np

sys.path.insert(0, "/opt/trn_rl_repo")

import ml_dtypes

N_CORES = 8
IN_DIM = 128
MEM = 128
L_GLOBAL = 262144
L = L_GLOBAL // N_CORES  # 32768 leaves per core
LOCAL_DEPTH = 15
DEVICE_DEPTH = 5  # device reduces to 1024 nodes/core; host does the rest
F = 2048  # chunk size along the node axis
FP8_LEVELS = (1, 2)  # gate GEMMs in fp8 DoubleRow at these levels
POOL_ADD_LEVELS = (1, 2, 3)  # c' adds on GpSimd here, on DVE above

_STATE = {}

LAST_EXEC_NS = None
LAST_RESULTS = None


def _build_module():
    import concourse.bacc as bacc
    import concourse.mybir as mybir
    import concourse.tile as tile

    bf = mybir.dt.bfloat16
    f8 = mybir.dt.float8e4
    f32 = mybir.dt.float32
    AF = mybir.ActivationFunctionType
    DR = mybir.MatmulPerfMode.DoubleRow

    nc = bacc.Bacc(
        "TRN2",
        target_bir_lowering=False,
        debug=False,
        enable_asserts=False,
    )

    # x8: leaf inputs, feature dim split across DoubleRow k-tiles:
    # x8[p, t, n] = x_bitrev[n, 64*t + p]
    x8 = nc.dram_tensor("x8", [64, 2, L], f8, kind="ExternalInput").ap()
    # leaf weights [64, 2, 128]: [p, t, m] = W.T[64*t + p, m]
    wcx8 = nc.dram_tensor("wcx8", [64, 2, 128], f8, kind="ExternalInput").ap()
    wox8 = nc.dram_tensor("wox8", [64, 2, 128], f8, kind="ExternalInput").ap()
    wl = nc.dram_tensor("wl", [128, 640], bf, kind="ExternalInput").ap()
    wr = nc.dram_tensor("wr", [128, 640], bf, kind="ExternalInput").ap()
    # fp8 DoubleRow gate weights: [:, g, 0, :] = Wl[g].T, [:, g, 1, :] = Wr[g].T
    w8 = nc.dram_tensor("w8", [128, 5, 2, 128], f8, kind="ExternalInput").ap()
    # bias columns: 0=bcx, 1=box, 2..6 = (bl+br)[gate] for gates i,lf,rf,o,u
    bv = nc.dram_tensor("bv", [128, 7], f32, kind="ExternalInput").ap()
    NOUT = L >> DEVICE_DEPTH
    out = nc.dram_tensor("out", [128, 2 * NOUT], f32, kind="ExternalOutput").ap()

    with tile.TileContext(nc) as tc:
        with (
            tc.tile_pool(name="const", bufs=1) as cpool,
            tc.tile_pool(name="levels", bufs=1) as lpool,
            tc.tile_pool(name="work", bufs=2) as wpool,
            tc.tile_pool(name="psum", bufs=2, space="PSUM") as ppool,
        ):
            wcx_t = cpool.tile([64, 2, 128], f8, name="wcx_t")
            nc.sync.dma_start(wcx_t, wcx8)
            wox_t = cpool.tile([64, 2, 128], f8, name="wox_t")
            nc.sync.dma_start(wox_t, wox8)
            wl_t = cpool.tile([128, 640], bf, name="wl_t")
            nc.sync.dma_start(wl_t, wl)
            wr_t = cpool.tile([128, 640], bf, name="wr_t")
            nc.sync.dma_start(wr_t, wr)
            w8_t = cpool.tile([128, 5, 2, 128], f8, name="w8_t")
            nc.sync.dma_start(w8_t, w8)
            bias_t = cpool.tile([128, 7], f32, name="bias_t")
            nc.sync.dma_start(bias_t, bv)

            # level buffers. h1 keeps the [128, 2, half] kt layout (it is the
            # fp8 DoubleRow rhs for level 2); everything else is flat.
            cb = {
                1: lpool.tile([128, L >> 1], bf, name="c1", tag="c_odd",
                              padded_shape=[128, L >> 1]),
                2: lpool.tile([128, L >> 2], bf, name="c2", tag="c_even",
                              padded_shape=[128, L >> 2]),
                3: lpool.tile([128, L >> 3], bf, name="c3", tag="c_odd",
                              padded_shape=[128, L >> 1]),
                4: lpool.tile([128, L >> 4], bf, name="c4", tag="c_even",
                              padded_shape=[128, L >> 2]),
            }
            hb = {
                1: lpool.tile([128, 2, L >> 2], f8, name="h1", tag="h_odd",
                              padded_shape=[128, 2, L >> 2]),
                2: lpool.tile([128, L >> 2], bf, name="h2", tag="h_even",
                              padded_shape=[128, L >> 2]),
                3: lpool.tile([128, L >> 3], bf, name="h3", tag="h_odd",
                              padded_shape=[128, L >> 2]),
                4: lpool.tile([128, L >> 4], bf, name="h4", tag="h_even",
                              padded_shape=[128, L >> 2]),
            }
            oc = lpool.tile([128, NOUT], f32, name="oc")
            oh = lpool.tile([128, NOUT], f32, name="oh")

            def mm_dr(gp, wtile, rhs2, f):
                """out = w.T@rhs over K=2x contraction, fp8 DoubleRow."""
                for s in range(0, f, 512):
                    e = min(s + 512, f)
                    nc.tensor.matmul(
                        gp[:, s:e], wtile, rhs2[:, :, s:e],
                        start=True, stop=True, perf_mode=DR,
                    )

            def mm_pair_bf16(gp, g, lh, rh, f):
                wls = wl_t[:, g * 128 : (g + 1) * 128]
                wrs = wr_t[:, g * 128 : (g + 1) * 128]
                for s in range(0, f, 512):
                    e = min(s + 512, f)
                    nc.tensor.matmul(
                        gp[:, s:e], wls, lh[:, s:e], start=True, stop=False
                    )
                    nc.tensor.matmul(
                        gp[:, s:e], wrs, rh[:, s:e], start=False, stop=True
                    )

            # ---- pending h spans: tanh(c')*o applied in batched passes ----
            # Spans are emitted "aged": a span completed during chunk c is
            # emitted at the start of chunk c+2's assembly, by which point its
            # c' (DVE muls -> GpSimd adds) is guaranteed done, so the in-order
            # ACT engine never head-of-line blocks on it. The emission point
            # (assembly start) is exactly the window where ACT would otherwise
            # idle on the h->GEMM->sigmoid dependency chain.
            og1_tiles = {}
            ready_q = []  # aged spans, emit at next assembly start
            fresh_q = []  # spans completed during the current chunk

            def emit_h_span(k, s, ln):
                if k == 1:
                    half = L >> 2
                    kt, off = (0, s) if s < half else (1, s - half)
                    csl = cb[1][:, s : s + ln]
                    tcy = wpool.tile([128, ln], bf, name="tcy", tag="tcy")
                    nc.scalar.activation(tcy, csl, AF.Tanh)
                    og = og1_tiles.pop(s)
                    nc.vector.tensor_mul(
                        hb[1][:, kt, off : off + ln], og, tcy
                    )
                else:
                    csl = cb[k][:, s : s + ln]
                    tcy = wpool.tile([128, ln], bf, name="tcy", tag="tcy")
                    nc.scalar.activation(tcy, csl, AF.Tanh)
                    hsl = hb[k][:, s : s + ln]
                    nc.vector.tensor_mul(hsl, hsl, tcy)

            def chunk_start():
                for sp in ready_q:
                    emit_h_span(*sp)
                ready_q[:] = fresh_q
                fresh_q[:] = []

            def flush_level(k):
                """Force-emit all pending spans of level k (required before
                level k+1's gate GEMMs may consume hb[k])."""
                for q in (ready_q, fresh_q):
                    for sp in [e for e in q if e[0] == k]:
                        emit_h_span(*sp)
                    q[:] = [e for e in q if e[0] != k]

            def h_pairs(X):
                HF = F
                if X >= 2 * HF:
                    return [(s, X // 2 + s, HF) for s in range(0, X // 2, HF)]
                return [(0, X // 2, X // 2)] if X >= 2 else [(0, 0, X)]

            # ---- per-level chunk emission ----
            def emit_level_chunk(k, f, rhs_dr, lh, rh, lc, rc, dst_c, dst_og,
                                 og_key):
                chunk_start()
                gps = []
                for g in range(5):
                    gp = ppool.tile([128, f], f32, name=f"g{g}", tag="ps")
                    if k in FP8_LEVELS:
                        mm_dr(gp, w8_t[:, g], rhs_dr, f)
                    else:
                        mm_pair_bf16(gp, g, lh, rh, f)
                    gps.append(gp)
                it = wpool.tile([128, f], bf, name="it", tag="it")
                nc.scalar.activation(it, gps[0], AF.Sigmoid, bias=bias_t[:, 2:3])
                lf_ = wpool.tile([128, f], bf, name="lf_", tag="lf_")
                nc.scalar.activation(lf_, gps[1], AF.Sigmoid, bias=bias_t[:, 3:4])
                rf_ = wpool.tile([128, f], bf, name="rf_", tag="rf_")
                nc.scalar.activation(rf_, gps[2], AF.Sigmoid, bias=bias_t[:, 4:5])
                if og_key is not None:
                    og = wpool.tile([128, f], bf, name="og1", tag="og1", bufs=4)
                    og1_tiles[og_key] = og
                    nc.scalar.activation(og, gps[3], AF.Sigmoid, bias=bias_t[:, 5:6])
                else:
                    nc.scalar.activation(
                        dst_og, gps[3], AF.Sigmoid, bias=bias_t[:, 5:6]
                    )
                ut = wpool.tile([128, f], bf, name="ut", tag="ut")
                nc.scalar.activation(ut, gps[4], AF.Tanh, bias=bias_t[:, 6:7])
                nc.vector.tensor_mul(it, it, ut)     # i*u
                nc.vector.tensor_mul(lf_, lf_, lc)   # lf*lc
                nc.vector.tensor_mul(rf_, rf_, rc)   # rf*rc
                if k in POOL_ADD_LEVELS:
                    nc.gpsimd.tensor_add(it, it, lf_)
                    nc.gpsimd.tensor_add(dst_c, it, rf_)
                else:
                    nc.vector.tensor_add(it, it, lf_)
                    nc.vector.tensor_add(dst_c, it, rf_)

            # ---- fused leaf + level-1 pass ----
            half1 = L >> 1  # 16384 parents at level 1
            X1h = half1 // 2
            l1_order = []
            for s in range(0, X1h, F):
                l1_order += [s, X1h + s]
            pairs1 = h_pairs(half1)

            def emit_leaf_pair(j):
                """leaf transform for leaf chunks [j, j+F) (left children)
                and [half1+j, ...) (right children) of L1 chunk j."""
                xt_l = wpool.tile([64, 2, 2 * F], f8, name="xt_l", tag="xt_l",
                                  bufs=2)
                nc.sync.dma_start(xt_l[:, :, 0:F], x8[:, :, j : j + F])
                nc.sync.dma_start(
                    xt_l[:, :, F : 2 * F], x8[:, :, half1 + j : half1 + j + F]
                )
                cl2 = wpool.tile([128, 2, F], bf, name="cl2", tag="cl2", bufs=2)
                hl2 = wpool.tile([128, 2, F], f8, name="hl2", tag="hl2", bufs=2)
                for c in range(2):
                    xs = xt_l[:, :, c * F : (c + 1) * F]
                    pc = ppool.tile([128, F], f32, name="pc", tag="ps")
                    mm_dr(pc, wcx_t, xs, F)
                    po = ppool.tile([128, F], f32, name="po", tag="ps")
                    mm_dr(po, wox_t, xs, F)
                    th = wpool.tile([128, F], bf, name="th", tag="th")
                    nc.scalar.activation(th, pc, AF.Tanh, bias=bias_t[:, 0:1])
                    og = wpool.tile([128, F], bf, name="og0", tag="og0")
                    nc.scalar.activation(og, po, AF.Sigmoid, bias=bias_t[:, 1:2])
                    # hl first: it gates the next chunk's GEMMs; cl is not
                    # needed until the assembly's DVE products
                    nc.vector.tensor_mul(hl2[:, c], og, th)
                    nc.vector.tensor_scalar_add(cl2[:, c], pc, bias_t[:, 0:1])
                return cl2, hl2

            hi1 = 0
            done1 = set()

            def drain1():
                nonlocal hi1
                while hi1 < len(pairs1):
                    s1, s2, ln = pairs1[hi1]
                    if not (s1 in done1 and s2 in done1):
                        break
                    fresh_q.append((1, s1, ln))
                    fresh_q.append((1, s2, ln))
                    hi1 += 1

            def l1_assembly(prev):
                (cl2, hl2), pj = prev
                emit_level_chunk(
                    1, F, hl2, None, None, cl2[:, 0], cl2[:, 1],
                    cb[1][:, pj : pj + F], None, og_key=pj,
                )
                done1.add(pj)
                drain1()

            prev = None
            for j in l1_order:
                if prev is not None:
                    l1_assembly(prev)
                prev = (emit_leaf_pair(j), j)
            l1_assembly(prev)

            # ---- levels 2..DEVICE_DEPTH ----
            for k in range(2, DEVICE_DEPTH + 1):
                flush_level(k - 1)
                X = L >> k  # parents at this level
                Xh = X // 2
                f = min(F, X)
                pairs = h_pairs(X)
                hi = 0
                if X // f >= 2:
                    order = []
                    for a, b in zip(range(0, Xh, f), range(Xh, X, f)):
                        order += [a, b]
                else:
                    order = [0]
                done = set()

                def span_ready(s, ln, done=done, f=f):
                    return all(q - q % f in done for q in range(s, s + ln, f))

                for j in order:
                    if k == 2:
                        rhs_dr = hb[1][:, :, j : j + f]
                        lh = rh = None
                    else:
                        rhs_dr = None
                        lh = hb[k - 1][:, j : j + f]
                        rh = hb[k - 1][:, X + j : X + j + f]
                    lc = cb[k - 1][:, j : j + f]
                    rc = cb[k - 1][:, X + j : X + j + f]
                    if k == DEVICE_DEPTH:
                        dst_c = oc[:, j : j + f]
                        og = wpool.tile([128, f], bf, name="ogN", tag="ogN", bufs=1)
                        emit_level_chunk(
                            k, f, rhs_dr, lh, rh, lc, rc, dst_c, og,
                            og_key=None,
                        )
                        tcy = wpool.tile([128, f], bf, name="tcyN", tag="tcy")
                        nc.scalar.activation(tcy, dst_c, AF.Tanh)
                        nc.vector.tensor_mul(oh[:, j : j + f], og, tcy)
                    else:
                        dst_c = cb[k][:, j : j + f]
                        dst_og = hb[k][:, j : j + f]
                        emit_level_chunk(
                            k, f, rhs_dr, lh, rh, lc, rc, dst_c, dst_og,
                            og_key=None,
                        )
                        done.add(j)
                        while hi < len(pairs):
                            s1, s2, ln = pairs[hi]
                            if not (span_ready(s1, ln) and span_ready(s2, ln)):
                                break
                            fresh_q.append((k, s1, ln))
                            if s2 > s1:
                                fresh_q.append((k, s2, ln))
                            hi += 1

            for kk in range(1, DEVICE_DEPTH):
                flush_level(kk)
            nc.sync.dma_start(out[:, 0:NOUT], oc)
            nc.sync.dma_start(out[:, NOUT : 2 * NOUT], oh)

    nc.compile()
    return nc


def _get_module():
    if "nc" not in _STATE:
        _STATE["nc"] = _build_module()
    return _STATE["nc"]


def _bitrev_perm(bits):
    n = 1 << bits
    i = np.arange(n, dtype=np.int64)
    r = np.zeros_like(i)
    for b in range(bits):
        r |= ((i >> b) & 1) << (bits - 1 - b)
    return r


def _run_spmd(nc, in_maps, trace):
    """Run via run_bass_kernel_spmd; with trace, drive NTFF profiling
    directly."""
    from concourse import bass_utils

    if not trace:
        res = bass_utils.run_bass_kernel_spmd(
            nc, in_maps, core_ids=list(range(N_CORES))
        )
        return res.results, None, None

    import glob
    import tempfile

    from concourse import bass2jax

    hook = None
    try:
        from trn_agent_boot.trn_boot import _ntff_profile_via_ctypes

        hook = _ntff_profile_via_ctypes("/opt/axon/libaxon_pjrt.so")
    except Exception as e:  # noqa: BLE001
        print(f"trace hook unavailable: {e}")
    if hook is None:
        res = bass_utils.run_bass_kernel_spmd(
            nc, in_maps, core_ids=list(range(N_CORES))
        )
        return res.results, None, None

    neff_dir = tempfile.mkdtemp(prefix="bk_prof_")
    with hook(neff_dir, [0]):
        results = bass2jax.run_bass_via_pjrt(nc, in_maps, n_cores=N_CORES)

    exec_ns = None
    trace_path = None
    ntffs = glob.glob(os.path.join(neff_dir, "*_body*.ntff"))
    if ntffs:
        try:
            import gauge.profiler as gp
            from concourse._compat import FishPath

            profile = gp.Profile(
                profile_path=FishPath(neff_dir),
                kernel_dev_mode=True,
                profile_on_exit=False,
                bass_kernel=nc.m,
                offline_processing=True,
                fname="*_body*",
            )
            prs = profile.to_perfetto(model_index=(0,))
            if prs:
                exec_ns = prs[0].exec_time_ns
                trace_path = prs[0].trace_path
        except Exception as e:  # noqa: BLE001
            print(f"ntff processing failed: {e}")
    else:
        print(f"no NTFF produced in {neff_dir}")
    return results, exec_ns, (neff_dir, trace_path)


def kernel(inputs, Wcx, bcx, Wox, box, Wl, bl, Wr, br):
    global LAST_EXEC_NS, LAST_RESULTS

    fp8 = ml_dtypes.float8_e4m3fn
    bf16 = ml_dtypes.bfloat16
    x = np.asarray(inputs, np.float32)
    Wcx = np.asarray(Wcx, np.float32)
    bcx = np.asarray(bcx, np.float32)
    Wox = np.asarray(Wox, np.float32)
    box = np.asarray(box, np.float32)
    Wl = np.asarray(Wl, np.float32)
    bl = np.asarray(bl, np.float32)
    Wr = np.asarray(Wr, np.float32)
    br = np.asarray(br, np.float32)

    nc = _get_module()

    # leaf weights [64, 2, 128]: [p, t, m] = W.T[64t+p, m]
    Wcx8 = np.ascontiguousarray(Wcx.T.reshape(2, 64, 128).transpose(1, 0, 2)).astype(fp8)
    Wox8 = np.ascontiguousarray(Wox.T.reshape(2, 64, 128).transpose(1, 0, 2)).astype(fp8)
    WlT = np.ascontiguousarray(
        np.concatenate([Wl[g].T for g in range(5)], axis=1)
    ).astype(bf16)  # [128, 640]
    WrT = np.ascontiguousarray(
        np.concatenate([Wr[g].T for g in range(5)], axis=1)
    ).astype(bf16)
    W8 = np.ascontiguousarray(
        np.stack(
            [np.stack([Wl[g].T, Wr[g].T], axis=1) for g in range(5)], axis=1
        )
    ).astype(fp8)  # [128, 5, 2, 128]
    bg = bl + br  # [5, 128]
    bvec = np.stack(
        [bcx, box, bg[0], bg[1], bg[2], bg[3], bg[4]], axis=1
    ).astype(np.float32)  # [128, 7]

    perm = _bitrev_perm(LOCAL_DEPTH)
    in_maps = []
    for m in range(N_CORES):
        shard = x[m * L : (m + 1) * L][perm]  # [L, 128]
        xt = np.ascontiguousarray(shard.T)  # [128, L] fp32
        x8v = np.ascontiguousarray(
            xt.reshape(2, 64, L).transpose(1, 0, 2)
        ).astype(fp8)  # [64, 2, L]
        in_maps.append(
            dict(x8=x8v, wcx8=Wcx8, wox8=Wox8, wl=WlT, wr=WrT, w8=W8, bv=bvec)
        )

    trace = bool(int(os.environ.get("BK_TRACE", "0")))
    results, exec_ns, trace_info = _run_spmd(nc, in_maps, trace)
    LAST_EXEC_NS = exec_ns
    LAST_RESULTS = trace_info

    bias5 = bg[:, None, :]  # [5, 1, 128]
    sig = lambda v: 1.0 / (1.0 + np.exp(-v))

    def level_np(c, h, lc, rc, lh, rh):
        g = (
            np.einsum("xm,gnm->gxn", lh, Wl)
            + np.einsum("xm,gnm->gxn", rh, Wr)
            + bias5
        )
        i = sig(g[0])
        lf = sig(g[1])
        rf = sig(g[2])
        o = sig(g[3])
        u = np.tanh(g[4])
        c = i * u + lf * lc + rf * rc
        h = o * np.tanh(c)
        return c, h

    NOUT = L >> DEVICE_DEPTH
    roots_c, roots_h = [], []
    for o in results:
        om = np.asarray(o["out"], np.float32)
        c = om[:, 0:NOUT].T  # [NOUT, 128]
        h = om[:, NOUT : 2 * NOUT].T
        while c.shape[0] > 1:
            half = c.shape[0] // 2
            c, h = level_np(c, h, c[:half], c[half:], h[:half], h[half:])
        roots_c.append(c[0])
        roots_h.append(h[0])
    c = np.stack(roots_c)  # [8, 128]
    h = np.stack(roots_h)
    while c.shape[0] > 1:
        c, h = level_np(c, h, c[0::2], c[1::2], h[0::2], h[1::2])
    return np.asarray(c, np.float32), np.asarray(h, np.float32)


# revision 21
# speedup vs baseline: 1.2849x; 1.0804x over previous
"""BinaryTreeLSTM forward on 8 Trainium2 NeuronCores.

Strategy
--------
Data-parallel over the leaf axis: each of the 8 cores takes a contiguous
block of 2^15 = 32768 leaves and reduces its subtree through level 5
(1024 nodes) on-chip; the host finishes the latency-bound tail (the
remaining local levels plus the 3 cross-core levels, ~8k of 262143
nodes) in fp32 during gather/unshard.

Layout: feature-on-partition. Leaves are permuted host-side by 15-bit
bit-reversal so at every level left children are the first half of the
node axis and right children the second half.

Engine split (the scalar/ACT engine is the roofline at ~240us busy):
 - PE: leaf + levels 1-2 gate GEMMs as fp8e4m3 DoubleRow matmuls
   (K=2x contraction in one instruction at 0.5 cyc/row); levels 3-5
   bf16. Cuts PE cycles ~2.5x so the PE p-state clock throttle cannot
   make PE the critical path.
 - ACT: all sigmoid/tanh exact, per-gate ops over [128, <=2048] spans.
 - DVE: gate products (bf16 2x rate), leaf c/h, h=o*tanh(c) writes
   (fp8 out for h feeding the fp8 levels), c' adds at the top levels.
 - GpSimd: the two c' accumulation adds at the wide levels.

Precision (validated in numpy emulation against the fp32 reference):
bf16 + fp8 leaf/L1/L2 GEMMs => ~8.6e-3 rel err (gate: 2e-2).
"""

import os
import sys

import numpy as np

sys.path.insert(0, "/opt/trn_rl_repo")

import ml_dtypes

N_CORES = 8
IN_DIM = 128
MEM = 128
L_GLOBAL = 262144
L = L_GLOBAL // N_CORES  # 32768 leaves per core
LOCAL_DEPTH = 15
DEVICE_DEPTH = 5  # device reduces to 1024 nodes/core; host does the rest
F = 2048  # chunk size along the node axis
FP8_LEVELS = (1, 2)  # gate GEMMs in fp8 DoubleRow at these levels
POOL_ADD_LEVELS = (1, 2, 3)  # c' adds on GpSimd here, on DVE above

_STATE = {}

LAST_EXEC_NS = None
LAST_RESULTS = None


def _build_module():
    import concourse.bacc as bacc
    import concourse.mybir as mybir
    import concourse.tile as tile

    bf = mybir.dt.bfloat16
    f8 = mybir.dt.float8e4
    f32 = mybir.dt.float32
    AF = mybir.ActivationFunctionType
    DR = mybir.MatmulPerfMode.DoubleRow

    nc = bacc.Bacc(
        "TRN2",
        target_bir_lowering=False,
        debug=False,
        enable_asserts=False,
    )

    # x8: leaf inputs, feature dim split across DoubleRow k-tiles:
    # x8[p, t, n] = x_bitrev[n, 64*t + p]
    x8 = nc.dram_tensor("x8", [64, 2, L], f8, kind="ExternalInput").ap()
    # leaf weights [64, 2, 128]: [p, t, m] = W.T[64*t + p, m]
    wcx8 = nc.dram_tensor("wcx8", [64, 2, 128], f8, kind="ExternalInput").ap()
    wox8 = nc.dram_tensor("wox8", [64, 2, 128], f8, kind="ExternalInput").ap()
    wl = nc.dram_tensor("wl", [128, 640], bf, kind="ExternalInput").ap()
    wr = nc.dram_tensor("wr", [128, 640], bf, kind="ExternalInput").ap()
    # fp8 DoubleRow gate weights: [:, g, 0, :] = Wl[g].T, [:, g, 1, :] = Wr[g].T
    w8 = nc.dram_tensor("w8", [128, 5, 2, 128], f8, kind="ExternalInput").ap()
    # bias columns: 0=bcx, 1=box, 2..6 = (bl+br)[gate] for gates i,lf,rf,o,u
    bv = nc.dram_tensor("bv", [128, 7], f32, kind="ExternalInput").ap()
    NOUT = L >> DEVICE_DEPTH
    out = nc.dram_tensor("out", [128, 2 * NOUT], f32, kind="ExternalOutput").ap()

    with tile.TileContext(nc) as tc:
        with (
            tc.tile_pool(name="const", bufs=1) as cpool,
            tc.tile_pool(name="levels", bufs=1) as lpool,
            tc.tile_pool(name="work", bufs=2) as wpool,
            tc.tile_pool(name="psum", bufs=2, space="PSUM") as ppool,
        ):
            wcx_t = cpool.tile([64, 2, 128], f8, name="wcx_t")
            nc.sync.dma_start(wcx_t, wcx8)
            wox_t = cpool.tile([64, 2, 128], f8, name="wox_t")
            nc.sync.dma_start(wox_t, wox8)
            wl_t = cpool.tile([128, 640], bf, name="wl_t")
            nc.sync.dma_start(wl_t, wl)
            wr_t = cpool.tile([128, 640], bf, name="wr_t")
            nc.sync.dma_start(wr_t, wr)
            w8_t = cpool.tile([128, 5, 2, 128], f8, name="w8_t")
            nc.sync.dma_start(w8_t, w8)
            bias_t = cpool.tile([128, 7], f32, name="bias_t")
            nc.sync.dma_start(bias_t, bv)

            # level buffers. h1 keeps the [128, 2, half] kt layout (it is the
            # fp8 DoubleRow rhs for level 2); everything else is flat.
            cb = {
                1: lpool.tile([128, L >> 1], bf, name="c1", tag="c_odd",
                              padded_shape=[128, L >> 1]),
                2: lpool.tile([128, L >> 2], bf, name="c2", tag="c_even",
                              padded_shape=[128, L >> 2]),
                3: lpool.tile([128, L >> 3], bf, name="c3", tag="c_odd",
                              padded_shape=[128, L >> 1]),
                4: lpool.tile([128, L >> 4], bf, name="c4", tag="c_even",
                              padded_shape=[128, L >> 2]),
            }
            hb = {
                1: lpool.tile([128, 2, L >> 2], f8, name="h1", tag="h_odd",
                              padded_shape=[128, 2, L >> 2]),
                2: lpool.tile([128, L >> 2], bf, name="h2", tag="h_even",
                              padded_shape=[128, L >> 2]),
                3: lpool.tile([128, L >> 3], bf, name="h3", tag="h_odd",
                              padded_shape=[128, L >> 2]),
                4: lpool.tile([128, L >> 4], bf, name="h4", tag="h_even",
                              padded_shape=[128, L >> 2]),
            }
            oc = lpool.tile([128, NOUT], f32, name="oc")
            oh = lpool.tile([128, NOUT], f32, name="oh")

            def mm_dr(gp, wtile, rhs2, f):
                """out = w.T@rhs over K=2x contraction, fp8 DoubleRow."""
                for s in range(0, f, 512):
                    e = min(s + 512, f)
                    nc.tensor.matmul(
                        gp[:, s:e], wtile, rhs2[:, :, s:e],
                        start=True, stop=True, perf_mode=DR,
                    )

            def mm_pair_bf16(gp, g, lh, rh, f):
                wls = wl_t[:, g * 128 : (g + 1) * 128]
                wrs = wr_t[:, g * 128 : (g + 1) * 128]
                for s in range(0, f, 512):
                    e = min(s + 512, f)
                    nc.tensor.matmul(
                        gp[:, s:e], wls, lh[:, s:e], start=True, stop=False
                    )
                    nc.tensor.matmul(
                        gp[:, s:e], wrs, rh[:, s:e], start=False, stop=True
                    )

            # ---- pending h spans: tanh(c')*o applied in batched passes ----
            # Spans are emitted "aged": a span completed during chunk c is
            # emitted at the start of chunk c+2's assembly, by which point its
            # c' (DVE muls -> GpSimd adds) is guaranteed done, so the in-order
            # ACT engine never head-of-line blocks on it. The emission point
            # (assembly start) is exactly the window where ACT would otherwise
            # idle on the h->GEMM->sigmoid dependency chain.
            og1_tiles = {}
            ready_q = []  # aged spans, emit at next assembly start
            fresh_q = []  # spans completed during the current chunk

            def emit_h_span(k, s, ln):
                if k == 1:
                    half = L >> 2
                    kt, off = (0, s) if s < half else (1, s - half)
                    csl = cb[1][:, s : s + ln]
                    tcy = wpool.tile([128, ln], bf, name="tcy", tag="tcy")
                    nc.scalar.activation(tcy, csl, AF.Tanh)
                    og = og1_tiles.pop(s)
                    nc.vector.tensor_mul(
                        hb[1][:, kt, off : off + ln], og, tcy
                    )
                else:
                    csl = cb[k][:, s : s + ln]
                    tcy = wpool.tile([128, ln], bf, name="tcy", tag="tcy")
                    nc.scalar.activation(tcy, csl, AF.Tanh)
                    hsl = hb[k][:, s : s + ln]
                    nc.vector.tensor_mul(hsl, hsl, tcy)

            def chunk_start():
                for sp in ready_q:
                    emit_h_span(*sp)
                ready_q[:] = fresh_q
                fresh_q[:] = []

            def flush_level(k):
                """Force-emit all pending spans of level k (required before
                level k+1's gate GEMMs may consume hb[k])."""
                for q in (ready_q, fresh_q):
                    for sp in [e for e in q if e[0] == k]:
                        emit_h_span(*sp)
                    q[:] = [e for e in q if e[0] != k]

            def h_pairs(X):
                HF = F
                if X >= 2 * HF:
                    return [(s, X // 2 + s, HF) for s in range(0, X // 2, HF)]
                return [(0, X // 2, X // 2)] if X >= 2 else [(0, 0, X)]

            # ---- per-level chunk emission ----
            def emit_level_chunk(k, f, rhs_dr, lh, rh, lc, rc, dst_c, dst_og,
                                 og_key, fast_adds=False):
                chunk_start()
                gps = []
                for g in range(5):
                    gp = ppool.tile([128, f], f32, name=f"g{g}", tag="ps")
                    if k in FP8_LEVELS:
                        mm_dr(gp, w8_t[:, g], rhs_dr, f)
                    else:
                        mm_pair_bf16(gp, g, lh, rh, f)
                    gps.append(gp)
                it = wpool.tile([128, f], bf, name="it", tag="it")
                nc.scalar.activation(it, gps[0], AF.Sigmoid, bias=bias_t[:, 2:3])
                lf_ = wpool.tile([128, f], bf, name="lf_", tag="lf_")
                nc.scalar.activation(lf_, gps[1], AF.Sigmoid, bias=bias_t[:, 3:4])
                rf_ = wpool.tile([128, f], bf, name="rf_", tag="rf_")
                nc.scalar.activation(rf_, gps[2], AF.Sigmoid, bias=bias_t[:, 4:5])
                if og_key is not None:
                    og = wpool.tile([128, f], bf, name="og1", tag="og1", bufs=4)
                    og1_tiles[og_key] = og
                    nc.scalar.activation(og, gps[3], AF.Sigmoid, bias=bias_t[:, 5:6])
                else:
                    nc.scalar.activation(
                        dst_og, gps[3], AF.Sigmoid, bias=bias_t[:, 5:6]
                    )
                ut = wpool.tile([128, f], bf, name="ut", tag="ut")
                nc.scalar.activation(ut, gps[4], AF.Tanh, bias=bias_t[:, 6:7])
                nc.vector.tensor_mul(it, it, ut)     # i*u
                nc.vector.tensor_mul(lf_, lf_, lc)   # lf*lc
                nc.vector.tensor_mul(rf_, rf_, rc)   # rf*rc
                if k in POOL_ADD_LEVELS and not fast_adds:
                    nc.gpsimd.tensor_add(it, it, lf_)
                    nc.gpsimd.tensor_add(dst_c, it, rf_)
                else:
                    nc.vector.tensor_add(it, it, lf_)
                    nc.vector.tensor_add(dst_c, it, rf_)

            # ---- fused leaf + level-1 pass ----
            half1 = L >> 1  # 16384 parents at level 1
            X1h = half1 // 2
            l1_order = []
            for s in range(0, X1h, F):
                l1_order += [s, X1h + s]
            pairs1 = h_pairs(half1)

            def emit_leaf_pair(j):
                """leaf transform for leaf chunks [j, j+F) (left children)
                and [half1+j, ...) (right children) of L1 chunk j."""
                xt_l = wpool.tile([64, 2, 2 * F], f8, name="xt_l", tag="xt_l",
                                  bufs=2)
                nc.sync.dma_start(xt_l[:, :, 0:F], x8[:, :, j : j + F])
                nc.sync.dma_start(
                    xt_l[:, :, F : 2 * F], x8[:, :, half1 + j : half1 + j + F]
                )
                cl2 = wpool.tile([128, 2, F], bf, name="cl2", tag="cl2", bufs=2)
                hl2 = wpool.tile([128, 2, F], f8, name="hl2", tag="hl2", bufs=2)
                for c in range(2):
                    xs = xt_l[:, :, c * F : (c + 1) * F]
                    pc = ppool.tile([128, F], f32, name="pc", tag="ps")
                    mm_dr(pc, wcx_t, xs, F)
                    po = ppool.tile([128, F], f32, name="po", tag="ps")
                    mm_dr(po, wox_t, xs, F)
                    th = wpool.tile([128, F], bf, name="th", tag="th")
                    nc.scalar.activation(th, pc, AF.Tanh, bias=bias_t[:, 0:1])
                    og = wpool.tile([128, F], bf, name="og0", tag="og0")
                    nc.scalar.activation(og, po, AF.Sigmoid, bias=bias_t[:, 1:2])
                    # cl first: it frees pc's PSUM slot (hl waits on og anyway)
                    nc.vector.tensor_scalar_add(cl2[:, c], pc, bias_t[:, 0:1])
                    nc.vector.tensor_mul(hl2[:, c], og, th)
                return cl2, hl2

            hi1 = 0
            done1 = set()

            def drain1():
                nonlocal hi1
                while hi1 < len(pairs1):
                    s1, s2, ln = pairs1[hi1]
                    if not (s1 in done1 and s2 in done1):
                        break
                    fresh_q.append((1, s1, ln))
                    fresh_q.append((1, s2, ln))
                    hi1 += 1

            def l1_assembly(prev, fast_adds=False):
                (cl2, hl2), pj = prev
                emit_level_chunk(
                    1, F, hl2, None, None, cl2[:, 0], cl2[:, 1],
                    cb[1][:, pj : pj + F], None, og_key=pj,
                    fast_adds=fast_adds,
                )
                done1.add(pj)
                drain1()

            prev = None
            for idx, j in enumerate(l1_order):
                if prev is not None:
                    l1_assembly(prev, fast_adds=idx >= len(l1_order) - 1)
                prev = (emit_leaf_pair(j), j)
            l1_assembly(prev, fast_adds=True)

            # ---- levels 2..DEVICE_DEPTH ----
            for k in range(2, DEVICE_DEPTH + 1):
                flush_level(k - 1)
                X = L >> k  # parents at this level
                Xh = X // 2
                f = min(F, X)
                pairs = h_pairs(X)
                hi = 0
                if X // f >= 2:
                    order = []
                    for a, b in zip(range(0, Xh, f), range(Xh, X, f)):
                        order += [a, b]
                else:
                    order = [0]
                done = set()

                def span_ready(s, ln, done=done, f=f):
                    return all(q - q % f in done for q in range(s, s + ln, f))

                for oi, j in enumerate(order):
                    fast = oi >= len(order) - 2
                    if k == 2:
                        rhs_dr = hb[1][:, :, j : j + f]
                        lh = rh = None
                    else:
                        rhs_dr = None
                        lh = hb[k - 1][:, j : j + f]
                        rh = hb[k - 1][:, X + j : X + j + f]
                    lc = cb[k - 1][:, j : j + f]
                    rc = cb[k - 1][:, X + j : X + j + f]
                    if k == DEVICE_DEPTH:
                        dst_c = oc[:, j : j + f]
                        og = wpool.tile([128, f], bf, name="ogN", tag="ogN", bufs=1)
                        emit_level_chunk(
                            k, f, rhs_dr, lh, rh, lc, rc, dst_c, og,
                            og_key=None, fast_adds=fast,
                        )
                        tcy = wpool.tile([128, f], bf, name="tcyN", tag="tcy")
                        nc.scalar.activation(tcy, dst_c, AF.Tanh)
                        nc.vector.tensor_mul(oh[:, j : j + f], og, tcy)
                    else:
                        dst_c = cb[k][:, j : j + f]
                        dst_og = hb[k][:, j : j + f]
                        emit_level_chunk(
                            k, f, rhs_dr, lh, rh, lc, rc, dst_c, dst_og,
                            og_key=None, fast_adds=fast,
                        )
                        done.add(j)
                        while hi < len(pairs):
                            s1, s2, ln = pairs[hi]
                            if not (span_ready(s1, ln) and span_ready(s2, ln)):
                                break
                            fresh_q.append((k, s1, ln))
                            if s2 > s1:
                                fresh_q.append((k, s2, ln))
                            hi += 1

            for kk in range(1, DEVICE_DEPTH):
                flush_level(kk)
            nc.sync.dma_start(out[:, 0:NOUT], oc)
            nc.sync.dma_start(out[:, NOUT : 2 * NOUT], oh)

    nc.compile()
    return nc


def _get_module():
    if "nc" not in _STATE:
        _STATE["nc"] = _build_module()
    return _STATE["nc"]


def _bitrev_perm(bits):
    n = 1 << bits
    i = np.arange(n, dtype=np.int64)
    r = np.zeros_like(i)
    for b in range(bits):
        r |= ((i >> b) & 1) << (bits - 1 - b)
    return r


def _run_spmd(nc, in_maps, trace):
    """Run via run_bass_kernel_spmd; with trace, drive NTFF profiling
    directly."""
    from concourse import bass_utils

    if not trace:
        res = bass_utils.run_bass_kernel_spmd(
            nc, in_maps, core_ids=list(range(N_CORES))
        )
        return res.results, None, None

    import glob
    import tempfile

    from concourse import bass2jax

    hook = None
    try:
        from trn_agent_boot.trn_boot import _ntff_profile_via_ctypes

        hook = _ntff_profile_via_ctypes("/opt/axon/libaxon_pjrt.so")
    except Exception as e:  # noqa: BLE001
        print(f"trace hook unavailable: {e}")
    if hook is None:
        res = bass_utils.run_bass_kernel_spmd(
            nc, in_maps, core_ids=list(range(N_CORES))
        )
        return res.results, None, None

    neff_dir = tempfile.mkdtemp(prefix="bk_prof_")
    with hook(neff_dir, [0]):
        results = bass2jax.run_bass_via_pjrt(nc, in_maps, n_cores=N_CORES)

    exec_ns = None
    trace_path = None
    ntffs = glob.glob(os.path.join(neff_dir, "*_body*.ntff"))
    if ntffs:
        try:
            import gauge.profiler as gp
            from concourse._compat import FishPath

            profile = gp.Profile(
                profile_path=FishPath(neff_dir),
                kernel_dev_mode=True,
                profile_on_exit=False,
                bass_kernel=nc.m,
                offline_processing=True,
                fname="*_body*",
            )
            prs = profile.to_perfetto(model_index=(0,))
            if prs:
                exec_ns = prs[0].exec_time_ns
                trace_path = prs[0].trace_path
        except Exception as e:  # noqa: BLE001
            print(f"ntff processing failed: {e}")
    else:
        print(f"no NTFF produced in {neff_dir}")
    return results, exec_ns, (neff_dir, trace_path)


def kernel(inputs, Wcx, bcx, Wox, box, Wl, bl, Wr, br):
    global LAST_EXEC_NS, LAST_RESULTS

    fp8 = ml_dtypes.float8_e4m3fn
    bf16 = ml_dtypes.bfloat16
    x = np.asarray(inputs, np.float32)
    Wcx = np.asarray(Wcx, np.float32)
    bcx = np.asarray(bcx, np.float32)
    Wox = np.asarray(Wox, np.float32)
    box = np.asarray(box, np.float32)
    Wl = np.asarray(Wl, np.float32)
    bl = np.asarray(bl, np.float32)
    Wr = np.asarray(Wr, np.float32)
    br = np.asarray(br, np.float32)

    nc = _get_module()

    # leaf weights [64, 2, 128]: [p, t, m] = W.T[64t+p, m]
    Wcx8 = np.ascontiguousarray(Wcx.T.reshape(2, 64, 128).transpose(1, 0, 2)).astype(fp8)
    Wox8 = np.ascontiguousarray(Wox.T.reshape(2, 64, 128).transpose(1, 0, 2)).astype(fp8)
    WlT = np.ascontiguousarray(
        np.concatenate([Wl[g].T for g in range(5)], axis=1)
    ).astype(bf16)  # [128, 640]
    WrT = np.ascontiguousarray(
        np.concatenate([Wr[g].T for g in range(5)], axis=1)
    ).astype(bf16)
    W8 = np.ascontiguousarray(
        np.stack(
            [np.stack([Wl[g].T, Wr[g].T], axis=1) for g in range(5)], axis=1
        )
    ).astype(fp8)  # [128, 5, 2, 128]
    bg = bl + br  # [5, 128]
    bvec = np.stack(
        [bcx, box, bg[0], bg[1], bg[2], bg[3], bg[4]], axis=1
    ).astype(np.float32)  # [128, 7]

    perm = _bitrev_perm(LOCAL_DEPTH)
    in_maps = []
    for m in range(N_CORES):
        shard = x[m * L : (m + 1) * L][perm]  # [L, 128]
        xt = np.ascontiguousarray(shard.T)  # [128, L] fp32
        x8v = np.ascontiguousarray(
            xt.reshape(2, 64, L).transpose(1, 0, 2)
        ).astype(fp8)  # [64, 2, L]
        in_maps.append(
            dict(x8=x8v, wcx8=Wcx8, wox8=Wox8, wl=WlT, wr=WrT, w8=W8, bv=bvec)
        )

    trace = bool(int(os.environ.get("BK_TRACE", "0")))
    results, exec_ns, trace_info = _run_spmd(nc, in_maps, trace)
    LAST_EXEC_NS = exec_ns
    LAST_RESULTS = trace_info

    bias5 = bg[:, None, :]  # [5, 1, 128]
    sig = lambda v: 1.0 / (1.0 + np.exp(-v))

    def level_np(c, h, lc, rc, lh, rh):
        g = (
            np.einsum("xm,gnm->gxn", lh, Wl)
            + np.einsum("xm,gnm->gxn", rh, Wr)
            + bias5
        )
        i = sig(g[0])
        lf = sig(g[1])
        rf = sig(g[2])
        o = sig(g[3])
        u = np.tanh(g[4])
        c = i * u + lf * lc + rf * rc
        h = o * np.tanh(c)
        return c, h

    NOUT = L >> DEVICE_DEPTH
    roots_c, roots_h = [], []
    for o in results:
        om = np.asarray(o["out"], np.float32)
        c = om[:, 0:NOUT].T  # [NOUT, 128]
        h = om[:, NOUT : 2 * NOUT].T
        while c.shape[0] > 1:
            half = c.shape[0] // 2
            c, h = level_np(c, h, c[:half], c[half:], h[:half], h[half:])
        roots_c.append(c[0])
        roots_h.append(h[0])
    c = np.stack(roots_c)  # [8, 128]
    h = np.stack(roots_h)
    while c.shape[0] > 1:
        c, h = level_np(c, h, c[0::2], c[1::2], h[0::2], h[1::2])
    return np.asarray(c, np.float32), np.asarray(h, np.float32)
